# revision 1
# baseline (speedup 1.0000x reference)
"""GAT (graph attention) layer on 8 Trainium2 NeuronCores.

Reference math (per head h):
    Wh = x @ W[h];  f_src = Wh @ a_src[h];  f_dst = Wh @ a_dst[h]
    e[i,j] = leaky_relu(f_src[i] + f_dst[j], alpha)
    att = softmax(where(adj>0, e, -9e15), axis=j)
    out[:, h*D:(h+1)*D] = att @ Wh

Exact identity used:
    exp(leaky_relu(s)) = exp(alpha*s) * max(1, exp((1-alpha)*s))
    numerator n[i,j] = p_i * adj[i,j] * max(q_j, z[i,j]),
        z[i,j] = exp((1-alpha)*f_src_i + f_dst_j),
        p_i = exp(alpha*f_src_i), q_j = exp(alpha*f_dst_j).
    p_i cancels in the softmax ratio, so with nhat[j,i] = adj*max(q_j, z):
        out_h[i, :] = (sum_j nhat[j,i]*Wh[j,:]) / (sum_j nhat[j,i])

Distribution: destination rows i sharded 1024/core; host passes adj.T
column-slices so device tiles arrive in [j-partition, i-free] orientation.

Per-core pipeline:
  A) xT via PE transpose; [Wh | f] = xT.T @ [W4 | wtilde] (float32r);
     Wh_aug (+ones col) bf16 weights; q cols; Fb = f_src broadcast (bf16).
  B) per j-block: DMA adjT int32; GPSIMD convert->bf16; per head:
     ACT z = Exp((1-a)*Fb + f_dst_col); DVE nhat = (z max q_col)*adj (STT);
     PE psum[h][65,1024] += Wh_aug.T @ nhat
  C) per head: psum->SBUF, PE-transpose, divide by denominator col, DMA out.
"""

import os
import numpy as np
from contextlib import ExitStack

import concourse.bass as bass
import concourse.tile as tile
from concourse import bacc, mybir
from concourse.bass_utils import run_bass_kernel_spmd
from concourse.masks import make_identity

N = 8192
DIN = 256
DOUT = 64
H = 4
NCORES = 8
SL = N // NCORES          # 1024 i's per core
NB = N // 128             # 64 j-blocks
SB = SL // 128            # 8 i-blocks per core
ALPHA = 0.2
W4C = H * DOUT            # 256
WAUG = H * (DOUT + 1)     # 260

f32 = mybir.dt.float32
f32r = mybir.dt.float32r
bf16 = mybir.dt.bfloat16
i32 = mybir.dt.int32
EXP = mybir.ActivationFunctionType.Exp
COPY = mybir.ActivationFunctionType.Copy
MAX = mybir.AluOpType.max
MULT = mybir.AluOpType.mult

_CACHE = {}


NBLIM = int(os.environ.get("NBLIM", str(NB)))
SKIPA = os.environ.get("SKIPA", "0") == "1"


def _build_module():
    nc = bacc.Bacc("TRN2", target_bir_lowering=False, debug=False,
                   num_devices=NCORES)

    x_d = nc.dram_tensor("x_full", [N, DIN], f32, kind="ExternalInput").ap()
    xs_d = nc.dram_tensor("x_slice", [SL, DIN], f32, kind="ExternalInput").ap()
    w_d = nc.dram_tensor("w_all", [H, DIN, DOUT], f32, kind="ExternalInput").ap()
    a_d = nc.dram_tensor("a_all", [H, 2, DOUT], f32, kind="ExternalInput").ap()
    adjT_d = nc.dram_tensor("adjT_slice", [N, SL], i32, kind="ExternalInput").ap()
    out_d = nc.dram_tensor("out_slice", [SL, H * DOUT], f32, kind="ExternalOutput").ap()

    with tile.TileContext(nc) as tc, ExitStack() as ctx:
        # ---------------- persistent tiles ----------------
        persist = ctx.enter_context(tc.tile_pool(name="persist", bufs=1))
        what_sb = persist.tile([128, NB, WAUG], bf16)   # Wh_aug per j-block
        fb_sb = persist.tile([128, H, SL], bf16)        # f_src broadcast
        fall_sb = persist.tile([128, NB, 2 * H], f32)   # [f_src(4) | f_dst(4)]
        q_sb = persist.tile([128, H, NB], f32)          # exp(alpha*f_dst)
        ps_sb = persist.tile([DOUT + 1, H, SL], f32)    # phase C staging
        ident_sb = persist.tile([128, 128], f32)
        make_identity(nc, ident_sb[:])

        # ======================= PHASE A =======================
        with ExitStack() as actx:
            a1pool = actx.enter_context(tc.tile_pool(name="aphase1", bufs=1))

            # --- W4 and a columns ---
            w4_sb = a1pool.tile([128, 2, W4C], f32)   # [d-part, d-chunk, h*64+o]
            for h in range(H):
                nc.sync.dma_start(
                    w4_sb[:, :, h * DOUT:(h + 1) * DOUT],
                    w_d[h].rearrange("(c p) o -> p c o", p=128))
            # a vectors broadcast across partitions (DMA step-0 AP)
            a_bc = a1pool.tile([128, H, 2, DOUT], f32)
            nc.gpsimd.dma_start(
                a_bc[:],
                bass.AP(tensor=a_d.tensor, offset=a_d.offset,
                        ap=[[0, 128]] + list(a_d.ap)))

            # --- wtilde[d, (s h)] = sum_o W4[d, o]*a[o]  (DVE reduce) ---
            wf_r = a1pool.tile([128, 2, W4C + 8], f32r)  # [W4 | wtilde(src4,dst4)]
            nc.vector.tensor_copy(wf_r[:, :, 0:W4C], w4_sb[:])
            wtl_sb = a1pool.tile([128, 2, 2, H], f32)
            ttr_dump = a1pool.tile([128, DOUT], f32)
            for c in range(2):
                for s in range(2):
                    for h in range(H):
                        nc.vector.scalar_tensor_tensor(
                            out=ttr_dump[:],
                            in0=w4_sb[:, c, h * DOUT:(h + 1) * DOUT],
                            scalar=1.0,
                            in1=a_bc[:, h, s, :],
                            op0=MULT, op1=MULT,
                            accum_out=wtl_sb[:, c, s, h:h + 1])
            nc.vector.tensor_copy(
                wf_r[:, :, W4C:],
                wtl_sb[:].rearrange("p c s h -> p c (s h)"))

            # --- x -> xT -> [Wh | f] per n-block ---
            CB = 16
            with ExitStack() as sctx:
                apool = sctx.enter_context(tc.tile_pool(name="aphase", bufs=2))
                apsum = sctx.enter_context(
                    tc.tile_pool(name="apsum_x", bufs=2, space="PSUM"))
                for cb0 in ([] if SKIPA else range(0, NB, CB)):
                    x_chunk = apool.tile([128, CB, DIN], f32, tag="xchunk")
                    nc.sync.dma_start(
                        x_chunk[:],
                        x_d.rearrange("(b p) d -> p b d", p=128)[:, cb0:cb0 + CB, :])
                    xt_chunk = apool.tile([128, 2, CB, 128], f32r, tag="xtchunk")
                    for bi in range(CB):
                        for c in range(2):
                            trp = apsum.tile([128, 128], f32, tag="trp")
                            nc.tensor.transpose(
                                trp[:], x_chunk[:, bi, c * 128:(c + 1) * 128],
                                ident_sb[:])
                            nc.vector.tensor_copy(xt_chunk[:, c, bi, :], trp[:])
                    for bi in range(CB):
                        b = cb0 + bi
                        whf = apsum.tile([128, W4C + 8], f32, tag="whf")
                        for c in range(2):
                            nc.tensor.matmul(whf[:], xt_chunk[:, c, bi, :],
                                             wf_r[:, c, :],
                                             start=(c == 0), stop=(c == 1))
                        nc.scalar.activation(
                            out=what_sb[:, b, :]
                                .rearrange("p (h o) -> p h o", h=H)[:, :, 0:DOUT],
                            in_=whf[:, 0:W4C].rearrange("p (h o) -> p h o", h=H),
                            func=COPY)
                        nc.vector.tensor_copy(fall_sb[:, b, :], whf[:, W4C:])
            # ones columns of Wh_aug
            for h in range(H):
                nc.vector.memset(what_sb[:, :, h * (DOUT + 1) + DOUT], 1.0)
            # q columns
            for h in range(H):
                nc.scalar.activation(
                    out=q_sb[:, h, :], in_=fall_sb[:, :, H + h],
                    func=EXP, scale=ALPHA)

            # --- Fb: f_src broadcast for the core's i-range ---
            with ExitStack() as sctx:
                fpool = sctx.enter_context(tc.tile_pool(name="afb", bufs=2))
                fpsum = sctx.enter_context(
                    tc.tile_pool(name="apsum_f", bufs=2, space="PSUM"))
                xs_sb = a1pool.tile([128, SB, DIN], f32)
                nc.sync.dma_start(
                    xs_sb[:], xs_d.rearrange("(b p) d -> p b d", p=128))
                fs_sb = a1pool.tile([128, SB, 4], f32)
                for bi in range(SB):
                    whf_s = fpsum.tile([128, W4C + 8], f32, tag="whfs")
                    for c in range(2):
                        xst = fpsum.tile([128, 128], f32, tag="xst")
                        nc.tensor.transpose(
                            xst[:], xs_sb[:, bi, c * 128:(c + 1) * 128],
                            ident_sb[:])
                        xst_r = fpool.tile([128, 128], f32r, tag="xstr")
                        nc.vector.tensor_copy(xst_r[:], xst[:])
                        nc.tensor.matmul(
                            whf_s[:], xst_r[:], wf_r[:, c, :],
                            start=(c == 0), stop=(c == 1))
                    nc.vector.tensor_copy(fs_sb[:, bi, :],
                                          whf_s[:, W4C:W4C + 4])
                fs_sb2 = fs_sb[:].rearrange("p b h -> p (b h)")
                fsT = fpsum.tile([SB * 4, 128], f32, tag="fsT", bufs=1)
                nc.tensor.transpose(fsT[:], fs_sb2, ident_sb[:])
                fsT_sb = a1pool.tile([SB * 4, 128], f32)
                nc.vector.tensor_copy(fsT_sb[:], fsT[:])
                # selection matrices: sel[:, bi, h, :] is [32,128] with row
                # (bi*4+h) all ones -> sel.T @ fsT broadcasts f_src chunk bi
                # across all 128 partitions.
                sel_sb = a1pool.tile([SB * 4, SB, H, 128], f32)
                nc.gpsimd.memset(sel_sb[:], 0.0)
                # expr = -k + 4*bi + h ; fill 1.0 where expr == 0
                nc.gpsimd.affine_select(
                    out=sel_sb[:], in_=sel_sb[:],
                    compare_op=mybir.AluOpType.not_equal,
                    fill=1.0, base=0,
                    pattern=[[4, SB], [1, H], [0, 128]],
                    channel_multiplier=-1)
                for h in range(H):
                    fbp = fpsum.tile([128, SL], f32, tag="fbp", bufs=1)
                    for bi in range(SB):
                        nc.tensor.matmul(
                            fbp[:, bi * 128:(bi + 1) * 128],
                            sel_sb[:, bi, h, :], fsT_sb[:],
                            start=True, stop=True)
                    nc.vector.tensor_copy(fb_sb[:, h, :], fbp[:])

        # ======================= PHASE B =======================
        with ExitStack() as bctx:
            bpool = bctx.enter_context(tc.tile_pool(name="bphase", bufs=3))
            bpsum = bctx.enter_context(
                tc.tile_pool(name="bpsum", bufs=1, space="PSUM"))
            ps = [bpsum.tile([DOUT + 1, SL], f32, tag=f"acc{h}", name=f"acc{h}")
                  for h in range(H)]

            for jb in range(NBLIM):
                adj_i = bpool.tile([128, SL], i32, tag="adji")
                nc.sync.dma_start(adj_i[:], adjT_d[jb * 128:(jb + 1) * 128, :])
                adj_b = bpool.tile([128, SL], bf16, tag="adjb")
                nc.gpsimd.tensor_copy(adj_b[:], adj_i[:])
                for h in range(H):
                    z = bpool.tile([128, SL], bf16, tag="z")
                    nc.scalar.activation(
                        out=z[:], in_=fb_sb[:, h, :], func=EXP,
                        scale=(1.0 - ALPHA),
                        bias=fall_sb[:, jb, H + h:H + h + 1])
                    nh = bpool.tile([128, SL], bf16, tag="nh")
                    nc.vector.scalar_tensor_tensor(
                        out=nh[:], in0=z[:], scalar=q_sb[:, h, jb:jb + 1],
                        in1=adj_b[:], op0=MAX, op1=MULT)
                    for half in range(2):
                        nc.tensor.matmul(
                            ps[h][:, half * 512:(half + 1) * 512],
                            what_sb[:, jb, h * (DOUT + 1):(h + 1) * (DOUT + 1)],
                            nh[:, half * 512:(half + 1) * 512],
                            start=(jb == 0), stop=(jb == NBLIM - 1))

            for h in range(H):
                nc.vector.tensor_copy(ps_sb[:, h, :], ps[h][:])

        # ======================= PHASE C =======================
        with ExitStack() as cctx:
            c2pool = cctx.enter_context(tc.tile_pool(name="c2", bufs=2))
            cpsum = cctx.enter_context(
                tc.tile_pool(name="cpsum", bufs=2, space="PSUM"))
            for bi in range(SB):
                o_sb = c2pool.tile([128, H * DOUT], f32, tag="osb")
                for h in range(H):
                    pst = cpsum.tile([128, DOUT + 1], f32, tag="pst")
                    nc.tensor.transpose(
                        pst[:], ps_sb[:, h, bi * 128:(bi + 1) * 128],
                        ident_sb[0:DOUT + 1, 0:DOUT + 1])
                    rec = c2pool.tile([128, 1], f32, tag="rec")
                    nc.vector.reciprocal(rec[:], pst[:, DOUT:DOUT + 1])
                    nc.vector.tensor_scalar_mul(
                        o_sb[:, h * DOUT:(h + 1) * DOUT], pst[:, 0:DOUT], rec[:])
                nc.sync.dma_start(out_d[bi * 128:(bi + 1) * 128, :], o_sb[:])

    nc.compile()
    return nc


def kernel(x, adj, W, a_src, a_dst):
    x = np.ascontiguousarray(np.asarray(x, dtype=np.float32))
    adj = np.asarray(adj, dtype=np.int32)
    W = np.ascontiguousarray(np.asarray(W, dtype=np.float32))
    a_all = np.ascontiguousarray(
        np.stack([np.asarray(a_src, np.float32),
                  np.asarray(a_dst, np.float32)], axis=1))  # [H, 2, DOUT]
    adjT = np.ascontiguousarray(adj.T)

    if "nc" not in _CACHE:
        _CACHE["nc"] = _build_module()
    nc = _CACHE["nc"]

    in_maps = []
    for c in range(NCORES):
        sl = slice(c * SL, (c + 1) * SL)
        in_maps.append({
            "x_full": x,
            "x_slice": np.ascontiguousarray(x[sl]),
            "w_all": W,
            "a_all": a_all,
            "adjT_slice": np.ascontiguousarray(adjT[:, sl]),
        })
    res = run_bass_kernel_spmd(nc, in_maps, core_ids=list(range(NCORES)))
    out = np.concatenate([res.results[c]["out_slice"] for c in range(NCORES)],
                         axis=0)
    return out



# revision 3
# speedup vs baseline: 1.2085x; 1.2085x over previous
"""GAT (graph attention) layer on 8 Trainium2 NeuronCores.

Reference math (per head h):
    Wh = x @ W[h];  f_src = Wh @ a_src[h];  f_dst = Wh @ a_dst[h]
    e[i,j] = leaky_relu(f_src[i] + f_dst[j], alpha)
    att = softmax(where(adj>0, e, -9e15), axis=j)
    out[:, h*D:(h+1)*D] = att @ Wh

Exact identity used (p_i = exp(alpha*f_src_i) cancels in the softmax):
    exp(leaky_relu(s)) = exp(alpha*s) * max(1, exp((1-alpha)*s))
    nhat[j,i] = adj[j,i] * max(q_j, u_i * v_j)
      with u_i = exp((1-alpha)*f_src_i), v_j = exp(f_dst_j),
           q_j = exp(alpha*f_dst_j)
    out_h[i,:] = (sum_j nhat[j,i]*Wh[j,:]) / (sum_j nhat[j,i])

Inner loop per (j-block, head): ONE dual-scalar tensor_scalar
    t = max(U_h * v_j, q_j)     (DVE 4x mode, per-partition scalars)
then ONE 4-head-batched tensor_tensor multiply with the bf16 0/1
adjacency tile (DVE 2x mode) and the usual PSUM matmul accumulation.

Distribution: destination rows i sharded 1024/core; host passes adj.T
column-slices cast to bf16 so device tiles arrive in [j-partition,
i-free] orientation with no device-side convert.
"""

import os
import numpy as np
from contextlib import ExitStack

import concourse.bass as bass
import concourse.tile as tile
from concourse import bacc, mybir
from concourse.bass_utils import run_bass_kernel_spmd
from concourse.masks import make_identity

N = 8192
DIN = 256
DOUT = 64
H = 4
NCORES = 8
SL = N // NCORES          # 1024 i's per core
NB = N // 128             # 64 j-blocks
SB = SL // 128            # 8 i-blocks per core
ALPHA = 0.2
W4C = H * DOUT            # 256
WAUG = H * (DOUT + 1)     # 260

f32 = mybir.dt.float32
f32r = mybir.dt.float32r
bf16 = mybir.dt.bfloat16
i32 = mybir.dt.int32
EXP = mybir.ActivationFunctionType.Exp
COPY = mybir.ActivationFunctionType.Copy
MAX = mybir.AluOpType.max
MULT = mybir.AluOpType.mult

_CACHE = {}


NBLIM = int(os.environ.get("NBLIM", str(NB)))
SKIPA = os.environ.get("SKIPA", "0") == "1"
POOLH = int(os.environ.get("POOLH", "1"))   # heads whose TS-dual runs on Pool


def _build_module():
    nc = bacc.Bacc("TRN2", target_bir_lowering=False, debug=False,
                   num_devices=NCORES)

    x_d = nc.dram_tensor("x_full", [N, DIN], f32, kind="ExternalInput").ap()
    xs_d = nc.dram_tensor("x_slice", [SL, DIN], f32, kind="ExternalInput").ap()
    w_d = nc.dram_tensor("w_all", [H, DIN, DOUT], f32, kind="ExternalInput").ap()
    a_d = nc.dram_tensor("a_all", [H, 2, DOUT], f32, kind="ExternalInput").ap()
    adjT_d = nc.dram_tensor("adjT_slice", [N, SL], bf16, kind="ExternalInput").ap()
    out_d = nc.dram_tensor("out_slice", [SL, H * DOUT], f32, kind="ExternalOutput").ap()

    with tile.TileContext(nc) as tc, ExitStack() as ctx:
        # ---------------- persistent tiles ----------------
        persist = ctx.enter_context(tc.tile_pool(name="persist", bufs=1))
        what_sb = persist.tile([128, NB, WAUG], bf16)   # Wh_aug per j-block
        fb_sb = persist.tile([128, H, SL], bf16)        # f_src broadcast
        u_sb = persist.tile([128, H, SL], bf16)         # exp((1-a)*f_src) bcast
        fall_sb = persist.tile([128, NB, 2 * H], f32)   # [f_src(4) | f_dst(4)]
        q_sb = persist.tile([128, H, NB], f32)          # exp(alpha*f_dst)
        v_sb = persist.tile([128, H, NB], f32)          # exp(f_dst)
        ps_sb = persist.tile([DOUT + 1, H, SL], f32)    # phase C staging
        ident_sb = persist.tile([128, 128], f32)
        make_identity(nc, ident_sb[:])

        # ======================= PHASE A =======================
        with ExitStack() as actx:
            a1pool = actx.enter_context(tc.tile_pool(name="aphase1", bufs=1))

            # --- W4 and a columns ---
            w4_sb = a1pool.tile([128, 2, W4C], f32)   # [d-part, d-chunk, h*64+o]
            for h in range(H):
                nc.sync.dma_start(
                    w4_sb[:, :, h * DOUT:(h + 1) * DOUT],
                    w_d[h].rearrange("(c p) o -> p c o", p=128))
            # a vectors broadcast across partitions (DMA step-0 AP)
            a_bc = a1pool.tile([128, H, 2, DOUT], f32)
            nc.gpsimd.dma_start(
                a_bc[:],
                bass.AP(tensor=a_d.tensor, offset=a_d.offset,
                        ap=[[0, 128]] + list(a_d.ap)))

            # --- wtilde[d, (s h)] = sum_o W4[d, o]*a[o]  (DVE reduce) ---
            wf_r = a1pool.tile([128, 2, W4C + 8], f32r)  # [W4 | wtilde(src4,dst4)]
            nc.vector.tensor_copy(wf_r[:, :, 0:W4C], w4_sb[:])
            wtl_sb = a1pool.tile([128, 2, 2, H], f32)
            ttr_dump = a1pool.tile([128, DOUT], f32)
            for c in range(2):
                for s in range(2):
                    for h in range(H):
                        nc.vector.scalar_tensor_tensor(
                            out=ttr_dump[:],
                            in0=w4_sb[:, c, h * DOUT:(h + 1) * DOUT],
                            scalar=1.0,
                            in1=a_bc[:, h, s, :],
                            op0=MULT, op1=MULT,
                            accum_out=wtl_sb[:, c, s, h:h + 1])
            nc.vector.tensor_copy(
                wf_r[:, :, W4C:],
                wtl_sb[:].rearrange("p c s h -> p c (s h)"))

            # --- x -> xT -> [Wh | f] per n-block ---
            CB = 16
            with ExitStack() as sctx:
                apool = sctx.enter_context(tc.tile_pool(name="aphase", bufs=2))
                apsum = sctx.enter_context(
                    tc.tile_pool(name="apsum_x", bufs=2, space="PSUM"))
                for cb0 in ([] if SKIPA else range(0, NB, CB)):
                    x_chunk = apool.tile([128, CB, DIN], f32, tag="xchunk")
                    nc.sync.dma_start(
                        x_chunk[:],
                        x_d.rearrange("(b p) d -> p b d", p=128)[:, cb0:cb0 + CB, :])
                    xt_chunk = apool.tile([128, 2, CB, 128], f32r, tag="xtchunk")
                    for bi in range(CB):
                        for c in range(2):
                            trp = apsum.tile([128, 128], f32, tag="trp")
                            nc.tensor.transpose(
                                trp[:], x_chunk[:, bi, c * 128:(c + 1) * 128],
                                ident_sb[:])
                            nc.vector.tensor_copy(xt_chunk[:, c, bi, :], trp[:])
                    for bi in range(CB):
                        b = cb0 + bi
                        whf = apsum.tile([128, W4C + 8], f32, tag="whf")
                        for c in range(2):
                            nc.tensor.matmul(whf[:], xt_chunk[:, c, bi, :],
                                             wf_r[:, c, :],
                                             start=(c == 0), stop=(c == 1))
                        nc.scalar.activation(
                            out=what_sb[:, b, :]
                                .rearrange("p (h o) -> p h o", h=H)[:, :, 0:DOUT],
                            in_=whf[:, 0:W4C].rearrange("p (h o) -> p h o", h=H),
                            func=COPY)
                        nc.vector.tensor_copy(fall_sb[:, b, :], whf[:, W4C:])
            # ones columns of Wh_aug
            for h in range(H):
                nc.vector.memset(what_sb[:, :, h * (DOUT + 1) + DOUT], 1.0)
            # q, v columns: exp(alpha*f_dst), exp(f_dst)
            for h in range(H):
                nc.scalar.activation(
                    out=q_sb[:, h, :], in_=fall_sb[:, :, H + h],
                    func=EXP, scale=ALPHA)
                nc.scalar.activation(
                    out=v_sb[:, h, :], in_=fall_sb[:, :, H + h],
                    func=EXP, scale=1.0)

            # --- Fb: f_src broadcast for the core's i-range ---
            with ExitStack() as sctx:
                fpool = sctx.enter_context(tc.tile_pool(name="afb", bufs=2))
                fpsum = sctx.enter_context(
                    tc.tile_pool(name="apsum_f", bufs=2, space="PSUM"))
                xs_sb = a1pool.tile([128, SB, DIN], f32)
                nc.sync.dma_start(
                    xs_sb[:], xs_d.rearrange("(b p) d -> p b d", p=128))
                fs_sb = a1pool.tile([128, SB, 4], f32)
                for bi in range(SB):
                    whf_s = fpsum.tile([128, W4C + 8], f32, tag="whfs")
                    for c in range(2):
                        xst = fpsum.tile([128, 128], f32, tag="xst")
                        nc.tensor.transpose(
                            xst[:], xs_sb[:, bi, c * 128:(c + 1) * 128],
                            ident_sb[:])
                        xst_r = fpool.tile([128, 128], f32r, tag="xstr")
                        nc.vector.tensor_copy(xst_r[:], xst[:])
                        nc.tensor.matmul(
                            whf_s[:], xst_r[:], wf_r[:, c, :],
                            start=(c == 0), stop=(c == 1))
                    nc.vector.tensor_copy(fs_sb[:, bi, :],
                                          whf_s[:, W4C:W4C + 4])
                fs_sb2 = fs_sb[:].rearrange("p b h -> p (b h)")
                fsT = fpsum.tile([SB * 4, 128], f32, tag="fsT", bufs=1)
                nc.tensor.transpose(fsT[:], fs_sb2, ident_sb[:])
                fsT_sb = a1pool.tile([SB * 4, 128], f32)
                nc.vector.tensor_copy(fsT_sb[:], fsT[:])
                # selection matrices: sel[:, bi, h, :] is [32,128] with row
                # (bi*4+h) all ones -> sel.T @ fsT broadcasts f_src chunk bi
                # across all 128 partitions.
                sel_sb = a1pool.tile([SB * 4, SB, H, 128], f32)
                nc.gpsimd.memset(sel_sb[:], 0.0)
                # expr = -k + 4*bi + h ; fill 1.0 where expr == 0
                nc.gpsimd.affine_select(
                    out=sel_sb[:], in_=sel_sb[:],
                    compare_op=mybir.AluOpType.not_equal,
                    fill=1.0, base=0,
                    pattern=[[4, SB], [1, H], [0, 128]],
                    channel_multiplier=-1)
                for h in range(H):
                    fbp = fpsum.tile([128, SL], f32, tag="fbp", bufs=1)
                    for bi in range(SB):
                        nc.tensor.matmul(
                            fbp[:, bi * 128:(bi + 1) * 128],
                            sel_sb[:, bi, h, :], fsT_sb[:],
                            start=True, stop=True)
                    nc.vector.tensor_copy(fb_sb[:, h, :], fbp[:])
                    # U_h = exp((1-alpha) * f_src_i), broadcast tile
                    nc.scalar.activation(
                        out=u_sb[:, h, :], in_=fb_sb[:, h, :],
                        func=EXP, scale=(1.0 - ALPHA))

        # ======================= PHASE B =======================
        with ExitStack() as bctx:
            bpool = bctx.enter_context(tc.tile_pool(name="bphase", bufs=3))
            bpsum = bctx.enter_context(
                tc.tile_pool(name="bpsum", bufs=1, space="PSUM"))
            ps = [bpsum.tile([DOUT + 1, SL], f32, tag=f"acc{h}", name=f"acc{h}")
                  for h in range(H)]

            for jb in range(NBLIM):
                adj_b = bpool.tile([128, SL], bf16, tag="adjb")
                nc.sync.dma_start(adj_b[:], adjT_d[jb * 128:(jb + 1) * 128, :])
                t4 = bpool.tile([128, H, SL], bf16, tag="t4")
                for h in range(H):
                    eng = nc.gpsimd if h < POOLH else nc.vector
                    eng.tensor_scalar(
                        t4[:, h, :], u_sb[:, h, :],
                        v_sb[:, h, jb:jb + 1], q_sb[:, h, jb:jb + 1],
                        op0=MULT, op1=MAX)
                # batched 4-head mask multiply (adj broadcast via stride-0)
                nh4 = bpool.tile([128, H, SL], bf16, tag="nh4")
                adj_bc = bass.AP(
                    tensor=adj_b[:].tensor, offset=adj_b[:].offset,
                    ap=[list(adj_b[:].ap[0]), [0, H], [1, SL]])
                nc.vector.tensor_tensor(nh4[:], t4[:], adj_bc, op=MULT)
                for h in range(H):
                    for half in range(2):
                        nc.tensor.matmul(
                            ps[h][:, half * 512:(half + 1) * 512],
                            what_sb[:, jb, h * (DOUT + 1):(h + 1) * (DOUT + 1)],
                            nh4[:, h, half * 512:(half + 1) * 512],
                            start=(jb == 0), stop=(jb == NBLIM - 1))

            for h in range(H):
                nc.vector.tensor_copy(ps_sb[:, h, :], ps[h][:])

        # ======================= PHASE C =======================
        with ExitStack() as cctx:
            c2pool = cctx.enter_context(tc.tile_pool(name="c2", bufs=2))
            cpsum = cctx.enter_context(
                tc.tile_pool(name="cpsum", bufs=2, space="PSUM"))
            for bi in range(SB):
                o_sb = c2pool.tile([128, H * DOUT], f32, tag="osb")
                for h in range(H):
                    pst = cpsum.tile([128, DOUT + 1], f32, tag="pst")
                    nc.tensor.transpose(
                        pst[:], ps_sb[:, h, bi * 128:(bi + 1) * 128],
                        ident_sb[0:DOUT + 1, 0:DOUT + 1])
                    rec = c2pool.tile([128, 1], f32, tag="rec")
                    nc.vector.reciprocal(rec[:], pst[:, DOUT:DOUT + 1])
                    nc.vector.tensor_scalar_mul(
                        o_sb[:, h * DOUT:(h + 1) * DOUT], pst[:, 0:DOUT], rec[:])
                nc.sync.dma_start(out_d[bi * 128:(bi + 1) * 128, :], o_sb[:])

    nc.compile()
    return nc


def kernel(x, adj, W, a_src, a_dst):
    x = np.ascontiguousarray(np.asarray(x, dtype=np.float32))
    adj = np.asarray(adj, dtype=np.int32)
    W = np.ascontiguousarray(np.asarray(W, dtype=np.float32))
    a_all = np.ascontiguousarray(
        np.stack([np.asarray(a_src, np.float32),
                  np.asarray(a_dst, np.float32)], axis=1))  # [H, 2, DOUT]
    import ml_dtypes
    # bf16 cast of the 0/1 mask is exact
    adjT_bf16 = np.ascontiguousarray(adj.T).astype(ml_dtypes.bfloat16)

    if "nc" not in _CACHE:
        _CACHE["nc"] = _build_module()
    nc = _CACHE["nc"]

    in_maps = []
    for c in range(NCORES):
        sl = slice(c * SL, (c + 1) * SL)
        in_maps.append({
            "x_full": x,
            "x_slice": np.ascontiguousarray(x[sl]),
            "w_all": W,
            "a_all": a_all,
            "adjT_slice": np.ascontiguousarray(adjT_bf16[:, sl]),
        })
    res = run_bass_kernel_spmd(nc, in_maps, core_ids=list(range(NCORES)))
    out = np.concatenate([res.results[c]["out_slice"] for c in range(NCORES)],
                         axis=0)
    return out


# revision 8
# speedup vs baseline: 1.3423x; 1.1107x over previous
"""GAT (graph attention) layer on 8 Trainium2 NeuronCores.

Reference math (per head h):
    Wh = x @ W[h];  f_src = Wh @ a_src[h];  f_dst = Wh @ a_dst[h]
    e[i,j] = leaky_relu(f_src[i] + f_dst[j], alpha)
    att = softmax(where(adj>0, e, -9e15), axis=j)
    out[:, h*D:(h+1)*D] = att @ Wh

Exact identity used (p_i = exp(alpha*f_src_i) cancels in the softmax):
    exp(leaky_relu(s)) = exp(alpha*s) * max(1, exp((1-alpha)*s))
    nhat[j,i] = adj[j,i] * max(q_j, u_i * v_j)
      with u_i = exp((1-alpha)*f_src_i), v_j = exp(f_dst_j),
           q_j = exp(alpha*f_dst_j)
    out_h[i,:] = (sum_j nhat[j,i]*Wh[j,:]) / (sum_j nhat[j,i])

Inner loop per (j-block, head): ONE dual-scalar tensor_scalar
    t = max(U_h * v_j, q_j)     (DVE 4x mode, per-partition scalars)
then ONE 4-head-batched tensor_tensor multiply with the bf16 0/1
adjacency tile (DVE 2x mode) and the usual PSUM matmul accumulation.

Distribution: destination rows i sharded 1024/core; host passes adj.T
column-slices cast to bf16 so device tiles arrive in [j-partition,
i-free] orientation with no device-side convert.
"""

import os
import numpy as np
from contextlib import ExitStack

import concourse.bass as bass
import concourse.tile as tile
from concourse import bacc, mybir
from concourse.bass_utils import run_bass_kernel_spmd
from concourse.masks import make_identity

N = 8192
DIN = 256
DOUT = 64
H = 4
NCORES = 8
SL = N // NCORES          # 1024 i's per core
NB = N // 128             # 64 j-blocks
SB = SL // 128            # 8 i-blocks per core
ALPHA = 0.2
W4C = H * DOUT            # 256
WAUG = H * (DOUT + 1)     # 260

f32 = mybir.dt.float32
f32r = mybir.dt.float32r
bf16 = mybir.dt.bfloat16
i32 = mybir.dt.int32
EXP = mybir.ActivationFunctionType.Exp
COPY = mybir.ActivationFunctionType.Copy
MAX = mybir.AluOpType.max
MULT = mybir.AluOpType.mult

_CACHE = {}


NBLIM = int(os.environ.get("NBLIM", str(NB)))
SKIPA = os.environ.get("SKIPA", "0") == "1"
POOLH = int(os.environ.get("POOLH", "1"))   # heads whose TS-dual runs on Pool


def _build_module():
    nc = bacc.Bacc("TRN2", target_bir_lowering=False, debug=False,
                   num_devices=NCORES)

    xt_d = nc.dram_tensor("xT_full", [DIN, N], bf16, kind="ExternalInput").ap()
    xst_d = nc.dram_tensor("xT_slice", [DIN, SL], bf16, kind="ExternalInput").ap()
    w_d = nc.dram_tensor("w_all", [H, DIN, DOUT], f32, kind="ExternalInput").ap()
    a_d = nc.dram_tensor("a_all", [H, 2, DOUT], f32, kind="ExternalInput").ap()
    adjT_d = nc.dram_tensor("adjT_slice", [N, SL], bf16, kind="ExternalInput").ap()
    out_d = nc.dram_tensor("out_slice", [SL, H * DOUT], f32, kind="ExternalOutput").ap()

    with tile.TileContext(nc) as tc, ExitStack() as ctx:
        # ---------------- persistent tiles ----------------
        persist = ctx.enter_context(tc.tile_pool(name="persist", bufs=1))
        what_sb = persist.tile([128, NB, WAUG], bf16)   # Wh_aug per j-block
        fb_sb = persist.tile([128, H, SL], bf16)        # f_src broadcast
        u_sb = persist.tile([128, H, SL], bf16)         # exp((1-a)*f_src) bcast
        fall_sb = persist.tile([128, NB, 2 * H], f32)   # [f_src(4) | f_dst(4)]
        q_sb = persist.tile([128, H, NB], f32)          # exp(alpha*f_dst)
        v_sb = persist.tile([128, H, NB], f32)          # exp(f_dst)
        ps_sb = persist.tile([DOUT + 1, H, SL], f32)    # phase C staging
        ident_sb = persist.tile([128, 128], f32)
        make_identity(nc, ident_sb[:])

        # ======================= PHASE A =======================
        with ExitStack() as actx:
            a1pool = actx.enter_context(tc.tile_pool(name="aphase1", bufs=1))

            # --- W4 and a columns ---
            w4_sb = a1pool.tile([128, 2, W4C], f32)   # [d-part, d-chunk, h*64+o]
            for h in range(H):
                nc.sync.dma_start(
                    w4_sb[:, :, h * DOUT:(h + 1) * DOUT],
                    w_d[h].rearrange("(c p) o -> p c o", p=128))
            # a vectors broadcast across partitions (DMA step-0 AP)
            a_bc = a1pool.tile([128, H, 2, DOUT], f32)
            nc.gpsimd.dma_start(
                a_bc[:],
                bass.AP(tensor=a_d.tensor, offset=a_d.offset,
                        ap=[[0, 128]] + list(a_d.ap)))

            # --- wtilde[d, (s h)] = sum_o W4[d, o]*a[o]  (DVE reduce) ---
            wf_sb = a1pool.tile([128, 2, W4C + 8], f32)  # [W4 | wtilde(src4,dst4)]
            nc.vector.tensor_copy(wf_sb[:, :, 0:W4C], w4_sb[:])
            wtl_sb = a1pool.tile([128, 2, 2, H], f32)
            ttr_dump = a1pool.tile([128, DOUT], f32)
            for c in range(2):
                for s in range(2):
                    for h in range(H):
                        nc.vector.scalar_tensor_tensor(
                            out=ttr_dump[:],
                            in0=w4_sb[:, c, h * DOUT:(h + 1) * DOUT],
                            scalar=1.0,
                            in1=a_bc[:, h, s, :],
                            op0=MULT, op1=MULT,
                            accum_out=wtl_sb[:, c, s, h:h + 1])
            nc.vector.tensor_copy(
                wf_sb[:, :, W4C:],
                wtl_sb[:].rearrange("p c s h -> p c (s h)"))
            wf_b = a1pool.tile([128, 2, W4C + 8], bf16)
            nc.vector.tensor_copy(wf_b[:], wf_sb[:])

            # --- xT (host-transposed, bf16) -> [Wh | f] per n-block ---
            CB = 16
            with ExitStack() as sctx:
                apool = sctx.enter_context(tc.tile_pool(name="aphase", bufs=2))
                apsum = sctx.enter_context(
                    tc.tile_pool(name="apsum_x", bufs=2, space="PSUM"))
                for cb0 in ([] if SKIPA else range(0, NB, CB)):
                    xt_chunk = apool.tile([128, 2, CB * 128], bf16, tag="xtchunk")
                    nc.sync.dma_start(
                        xt_chunk[:],
                        xt_d.rearrange("(c p) n -> p c n", p=128)
                            [:, :, cb0 * 128:(cb0 + CB) * 128])
                    for bi in range(CB):
                        b = cb0 + bi
                        whf = apsum.tile([128, W4C + 8], f32, tag="whf")
                        for c in range(2):
                            nc.tensor.matmul(
                                whf[:],
                                xt_chunk[:, c, bi * 128:(bi + 1) * 128],
                                wf_b[:, c, :],
                                start=(c == 0), stop=(c == 1))
                        nc.scalar.activation(
                            out=what_sb[:, b, :]
                                .rearrange("p (h o) -> p h o", h=H)[:, :, 0:DOUT],
                            in_=whf[:, 0:W4C].rearrange("p (h o) -> p h o", h=H),
                            func=COPY)
                        nc.vector.tensor_copy(fall_sb[:, b, :], whf[:, W4C:])
            # ones columns of Wh_aug
            for h in range(H):
                nc.vector.memset(what_sb[:, :, h * (DOUT + 1) + DOUT], 1.0)
            # q, v columns: exp(alpha*f_dst), exp(f_dst)
            for h in range(H):
                nc.scalar.activation(
                    out=q_sb[:, h, :], in_=fall_sb[:, :, H + h],
                    func=EXP, scale=ALPHA)
                nc.scalar.activation(
                    out=v_sb[:, h, :], in_=fall_sb[:, :, H + h],
                    func=EXP, scale=1.0)

            # --- Fb: f_src broadcast for the core's i-range ---
            with ExitStack() as sctx:
                fpool = sctx.enter_context(tc.tile_pool(name="afb", bufs=2))
                fpsum = sctx.enter_context(
                    tc.tile_pool(name="apsum_f", bufs=2, space="PSUM"))
                xst_sb = a1pool.tile([128, 2, SL], bf16)
                nc.sync.dma_start(
                    xst_sb[:], xst_d.rearrange("(c p) n -> p c n", p=128))
                fs_sb = a1pool.tile([128, SB, 4], f32)
                for bi in range(SB):
                    whf_s = fpsum.tile([128, 4], f32, tag="whfs")
                    for c in range(2):
                        nc.tensor.matmul(
                            whf_s[:],
                            xst_sb[:, c, bi * 128:(bi + 1) * 128],
                            wf_b[:, c, W4C:W4C + 4],
                            start=(c == 0), stop=(c == 1))
                    nc.vector.tensor_copy(fs_sb[:, bi, :], whf_s[:])
                fs_sb2 = fs_sb[:].rearrange("p b h -> p (b h)")
                fsT = fpsum.tile([SB * 4, 128], f32, tag="fsT", bufs=1)
                nc.tensor.transpose(fsT[:], fs_sb2, ident_sb[:])
                fsT_sb = a1pool.tile([SB * 4, 128], f32)
                nc.vector.tensor_copy(fsT_sb[:], fsT[:])
                # selection matrices: sel[:, bi, h, :] is [32,128] with row
                # (bi*4+h) all ones -> sel.T @ fsT broadcasts f_src chunk bi
                # across all 128 partitions.
                sel_sb = a1pool.tile([SB * 4, SB, H, 128], f32)
                nc.gpsimd.memset(sel_sb[:], 0.0)
                # expr = -k + 4*bi + h ; fill 1.0 where expr == 0
                nc.gpsimd.affine_select(
                    out=sel_sb[:], in_=sel_sb[:],
                    compare_op=mybir.AluOpType.not_equal,
                    fill=1.0, base=0,
                    pattern=[[4, SB], [1, H], [0, 128]],
                    channel_multiplier=-1)
                for h in range(H):
                    fbp = fpsum.tile([128, SL], f32, tag="fbp", bufs=1)
                    for bi in range(SB):
                        nc.tensor.matmul(
                            fbp[:, bi * 128:(bi + 1) * 128],
                            sel_sb[:, bi, h, :], fsT_sb[:],
                            start=True, stop=True)
                    nc.vector.tensor_copy(fb_sb[:, h, :], fbp[:])
                    # U_h = exp((1-alpha) * f_src_i), broadcast tile
                    nc.scalar.activation(
                        out=u_sb[:, h, :], in_=fb_sb[:, h, :],
                        func=EXP, scale=(1.0 - ALPHA))

        # ======================= PHASE B =======================
        with ExitStack() as bctx:
            bpool = bctx.enter_context(tc.tile_pool(name="bphase", bufs=3))
            bpsum = bctx.enter_context(
                tc.tile_pool(name="bpsum", bufs=1, space="PSUM"))
            ps = [bpsum.tile([DOUT + 1, SL], f32, tag=f"acc{h}", name=f"acc{h}")
                  for h in range(H)]

            for jb in range(NBLIM):
                adj_b = bpool.tile([128, SL], bf16, tag="adjb")
                nc.sync.dma_start(adj_b[:], adjT_d[jb * 128:(jb + 1) * 128, :])
                t4 = bpool.tile([128, H, SL], bf16, tag="t4")
                for h in range(H):
                    eng = nc.gpsimd if h < POOLH else nc.vector
                    eng.tensor_scalar(
                        t4[:, h, :], u_sb[:, h, :],
                        v_sb[:, h, jb:jb + 1], q_sb[:, h, jb:jb + 1],
                        op0=MULT, op1=MAX)
                # batched 4-head mask multiply (adj broadcast via stride-0)
                nh4 = bpool.tile([128, H, SL], bf16, tag="nh4")
                adj_bc = bass.AP(
                    tensor=adj_b[:].tensor, offset=adj_b[:].offset,
                    ap=[list(adj_b[:].ap[0]), [0, H], [1, SL]])
                nc.vector.tensor_tensor(nh4[:], t4[:], adj_bc, op=MULT)
                for h in range(H):
                    for half in range(2):
                        nc.tensor.matmul(
                            ps[h][:, half * 512:(half + 1) * 512],
                            what_sb[:, jb, h * (DOUT + 1):(h + 1) * (DOUT + 1)],
                            nh4[:, h, half * 512:(half + 1) * 512],
                            start=(jb == 0), stop=(jb == NBLIM - 1))

            for h in range(H):
                nc.vector.tensor_copy(ps_sb[:, h, :], ps[h][:])

        # ======================= PHASE C =======================
        with ExitStack() as cctx:
            c2pool = cctx.enter_context(tc.tile_pool(name="c2", bufs=2))
            cpsum = cctx.enter_context(
                tc.tile_pool(name="cpsum", bufs=2, space="PSUM"))
            for bi in range(SB):
                o_sb = c2pool.tile([128, H * DOUT], f32, tag="osb")
                for h in range(H):
                    pst = cpsum.tile([128, DOUT + 1], f32, tag="pst")
                    nc.tensor.transpose(
                        pst[:], ps_sb[:, h, bi * 128:(bi + 1) * 128],
                        ident_sb[0:DOUT + 1, 0:DOUT + 1])
                    rec = c2pool.tile([128, 1], f32, tag="rec")
                    nc.vector.reciprocal(rec[:], pst[:, DOUT:DOUT + 1])
                    nc.vector.tensor_scalar_mul(
                        o_sb[:, h * DOUT:(h + 1) * DOUT], pst[:, 0:DOUT], rec[:])
                nc.sync.dma_start(out_d[bi * 128:(bi + 1) * 128, :], o_sb[:])

    nc.compile()
    return nc


def kernel(x, adj, W, a_src, a_dst):
    x = np.ascontiguousarray(np.asarray(x, dtype=np.float32))
    adj = np.asarray(adj, dtype=np.int32)
    W = np.ascontiguousarray(np.asarray(W, dtype=np.float32))
    a_all = np.ascontiguousarray(
        np.stack([np.asarray(a_src, np.float32),
                  np.asarray(a_dst, np.float32)], axis=1))  # [H, 2, DOUT]
    import ml_dtypes
    # bf16 cast of the 0/1 mask is exact
    adjT_bf16 = np.ascontiguousarray(adj.T).astype(ml_dtypes.bfloat16)
    xT_bf16 = np.ascontiguousarray(x.T.astype(ml_dtypes.bfloat16))

    if "nc" not in _CACHE:
        _CACHE["nc"] = _build_module()
    nc = _CACHE["nc"]

    in_maps = []
    for c in range(NCORES):
        sl = slice(c * SL, (c + 1) * SL)
        in_maps.append({
            "xT_full": xT_bf16,
            "xT_slice": np.ascontiguousarray(xT_bf16[:, sl]),
            "w_all": W,
            "a_all": a_all,
            "adjT_slice": np.ascontiguousarray(adjT_bf16[:, sl]),
        })
    res = run_bass_kernel_spmd(nc, in_maps, core_ids=list(range(NCORES)))
    out = np.concatenate([res.results[c]["out_slice"] for c in range(NCORES)],
                         axis=0)
    return out
